# revision 1
# baseline (speedup 1.0000x reference)
"""MultiHeadAttention Trainium2 kernel (8 NeuronCores, data-parallel over batch).

Contract: kernel(**inputs) takes the FULL inputs from setup_inputs() and
returns the FULL [8, 512, 1024] output. Internally, batch element c goes to
NeuronCore c (B == n_cores == 8); each core runs the same Bass/Tile program
on its own shard. No collectives needed.

Per-core computation (batch b, S=512, D=1024, H=16, Dk=64):
  QT = (w_q/8)^T-proj of query^T  -> [D, S]  (head h rows h*64..h*64+63)
  KT likewise (unscaled)          -> [D, S]
  V  = natural value proj         -> [S, D]  (stored with a ones column per head)
  per head: scoresT[k,q'] = KT_h-block^T @ QT_h   (q' = reversed query index)
            psum += amt[h]  (host-precomputed fp16 rel-bias + mask additive)
            attnT = exp(psum)                     (ScalarE, reads PSUM)
            ctxT[65,S] = [V_h | 1]^T @ attnT      (row 64 = softmax denominators)
            ctxT_norm = ctxT[0:64] * broadcast(1/denom)
  out_rev[q', e] = ctxT_norm^T-chunks @ w_o^T + b_o ; host un-reverses rows.

All matmuls run in fp32r (single-pass, 4x faster than fp32 LOW_HIGH mode);
matmul operands are pre-rounded to fp32r's 19-bit mantissa on the host (DMA
inputs) or rounded on write by ACT/DVE (on-chip producers), which the BIR
verifier requires. The query-direction reversal makes the relative-position
bias rel_bias[k - q + 511, h] equal rel_bias[k + q', h] — a positive-stride
layout the host can materialize directly.
"""
import numpy as np

import concourse.bass as bass
import concourse.tile as tile
from concourse import bacc, mybir
from concourse.bass_utils import run_bass_kernel_spmd

S = 512
D = 1024
H = 16
DK = 64
N_CORES = 8
NCH = D // 128  # 8 d-model chunks of 128
SB = S // 128   # 4 seq blocks of 128
F32 = mybir.dt.float32
F32R = mybir.dt.float32r
F16 = mybir.dt.float16

MASK_NEG = -30000.0  # large-negative additive mask, fp16-representable

_CACHE = {}


def _build_program():
    nc = bacc.Bacc("TRN2", target_bir_lowering=False, debug=False,
                   num_devices=N_CORES)

    # Per-core DRAM inputs (fp32r ones feed matmuls; host pre-rounds them)
    qT = nc.dram_tensor("qT", [D, S], F32R, kind="ExternalInput").ap()
    kT = nc.dram_tensor("kT", [D, S], F32R, kind="ExternalInput").ap()
    vT = nc.dram_tensor("vT", [D, S], F32R, kind="ExternalInput").ap()
    amt = nc.dram_tensor("amt", [H, S, S], F16, kind="ExternalInput").ap()
    wq = nc.dram_tensor("wq", [D, D], F32R, kind="ExternalInput").ap()
    wk = nc.dram_tensor("wk", [D, D], F32R, kind="ExternalInput").ap()
    wv = nc.dram_tensor("wv", [D, D], F32R, kind="ExternalInput").ap()
    wo = nc.dram_tensor("wo", [D, D], F32R, kind="ExternalInput").ap()
    bq = nc.dram_tensor("bq", [128, NCH], F32, kind="ExternalInput").ap()
    bk = nc.dram_tensor("bk", [128, NCH], F32, kind="ExternalInput").ap()
    bvr = nc.dram_tensor("bvr", [1, D], F32R, kind="ExternalInput").ap()
    bor = nc.dram_tensor("bor", [1, D], F32R, kind="ExternalInput").ap()
    out = nc.dram_tensor("out", [S, D], F32, kind="ExternalOutput").ap()

    # DRAM views for chunked DMA
    qT3 = qT.rearrange("(c p) s -> p c s", p=128)     # [128, 8, 512]
    kT3 = kT.rearrange("(c p) s -> p c s", p=128)
    vT3 = vT.rearrange("(c p) s -> p c s", p=128)
    amt4 = amt.rearrange("h (kb p) q -> h p kb q", p=128)  # [16, 128, 4, 512]
    wq3 = wq.rearrange("(c p) e -> c p e", p=128)     # [8, 128, 1024]
    wk3 = wk.rearrange("(c p) e -> c p e", p=128)
    wv3 = wv.rearrange("(c p) e -> c p e", p=128)
    wo3 = wo.rearrange("(c p) e -> c p e", p=128)
    out3 = out.rearrange("(sb p) e -> sb p e", p=128)  # [4, 128, 1024]

    from contextlib import ExitStack

    with tile.TileContext(nc) as tc, ExitStack() as ctx:
        singles = ctx.enter_context(tc.tile_pool(name="singles", bufs=1))
        wpool = ctx.enter_context(tc.tile_pool(name="wpool", bufs=16))
        rcpool = ctx.enter_context(tc.tile_pool(name="rcpool", bufs=2))
        rc1pool = ctx.enter_context(tc.tile_pool(name="rc1pool", bufs=1))
        ps_proj = ctx.enter_context(tc.tile_pool(name="ps_proj", bufs=2, space="PSUM"))
        ps_sc = ctx.enter_context(tc.tile_pool(name="ps_sc", bufs=3, space="PSUM"))
        ps_ctx = ctx.enter_context(tc.tile_pool(name="ps_ctx", bufs=2, space="PSUM"))
        ps_r = ctx.enter_context(tc.tile_pool(name="ps_r", bufs=1, space="PSUM"))
        vt_ctx = ExitStack()
        vtpool = vt_ctx.enter_context(tc.tile_pool(name="vtpool", bufs=1))

        # small constants first so the warm-up matmuls can start immediately
        bq_sb = singles.tile([128, NCH], F32, tag="bq")
        bk_sb = singles.tile([128, NCH], F32, tag="bk")
        nc.sync.dma_start(out=bq_sb, in_=bq)
        nc.sync.dma_start(out=bk_sb, in_=bk)
        bvr_sb = singles.tile([1, D], F32R, tag="bvr")
        bor_sb = singles.tile([1, D], F32R, tag="bor")
        nc.sync.dma_start(out=bvr_sb, in_=bvr)
        nc.sync.dma_start(out=bor_sb, in_=bor)
        # memset can't target fp32r; stage in fp32 and round via ACT copy
        ones_f32 = singles.tile([1, 128], F32, tag="ones_f32")
        nc.vector.memset(ones_f32, 1.0)
        ones_sb = singles.tile([1, 128], F32R, tag="ones")
        nc.scalar.copy(ones_sb, ones_f32)
        ones_col = singles.tile([128, H, 1], F32, tag="ones_col")
        nc.vector.memset(ones_col, 1.0)

        # HAM warm-up: ~5us of throwaway matmuls while the input DMAs stream,
        # so the PE clock-gate is at 8/8 by the time real work is ready.
        # Operands are built on-chip so no DMA gates the first matmul.
        for _ in range(40):
            pd = ps_proj.tile([128, 512], F32, tag="proj")
            nc.tensor.matmul(pd[:, :128], lhsT=ones_sb[:, :128], rhs=ones_sb,
                             start=True, stop=True)

        # bulk loads, emitted in consumption order (sync queue is FIFO)
        vT_sb = vtpool.tile([128, NCH, S], F32R, tag="vT")
        nc.sync.dma_start(out=vT_sb, in_=vT3)
        qT_sb = singles.tile([128, NCH, S], F32R, tag="qT")
        kT_sb = singles.tile([128, NCH, S], F32R, tag="kT")

        # big persistent activations
        QT_sb = singles.tile([128, NCH, S], F32R, tag="QT")
        KT_sb = singles.tile([128, NCH, S], F32R, tag="KT")
        # V with a ones column appended per head: [128, sb, 16*65]
        V_sb = singles.tile([128, SB, H * (DK + 1)], F32R, tag="V")
        ctxT_sb = singles.tile([128, NCH, S], F32R, tag="ctxT")

        # ---- V projection: V[s, e] = vT^T @ wvT + b_v ----
        wv_sb = []
        for dc in range(NCH):
            t = wpool.tile([128, D], F32R, tag="w")
            nc.sync.dma_start(out=t, in_=wv3[dc])
            wv_sb.append(t)
        nc.sync.dma_start(out=qT_sb, in_=qT3)
        nc.sync.dma_start(out=kT_sb, in_=kT3)
        for sb in range(SB):
            # set ones columns for this s-block
            v_heads = V_sb[:, sb, :].rearrange("p (h c) -> p h c", c=DK + 1)
            nc.scalar.copy(v_heads[:, :, DK:DK + 1], ones_col)
            for eh in range(2):
                pv = ps_proj.tile([128, 512], F32, tag="proj")
                for dc in range(NCH):
                    nc.tensor.matmul(
                        pv,
                        lhsT=vT_sb[:, dc, sb * 128:(sb + 1) * 128],
                        rhs=wv_sb[dc][:, eh * 512:(eh + 1) * 512],
                        start=(dc == 0), stop=False,
                    )
                nc.tensor.matmul(
                    pv, lhsT=ones_sb[:, :128],
                    rhs=bvr_sb[:, eh * 512:(eh + 1) * 512],
                    start=False, stop=True,
                )
                nc.scalar.copy(
                    v_heads[:, 8 * eh:8 * eh + 8, 0:DK],
                    pv.rearrange("p (h d) -> p h d", d=DK),
                )

        # vT no longer needed; free its SBUF for the pools below
        vt_ctx.close()
        amtpool = ctx.enter_context(tc.tile_pool(name="amtpool", bufs=2))
        attnpool = ctx.enter_context(tc.tile_pool(name="attnpool", bufs=9))
        outpool = ctx.enter_context(tc.tile_pool(name="outpool", bufs=2))

        # ---- interleaved Q/K projection chunks + attention heads ----
        wq_sb = []
        wk_sb = []
        for dc in range(NCH):
            t = wpool.tile([128, D], F32R, tag="w")
            nc.sync.dma_start(out=t, in_=wq3[dc])
            wq_sb.append(t)
        for dc in range(NCH):
            t = wpool.tile([128, D], F32R, tag="w")
            nc.sync.dma_start(out=t, in_=wk3[dc])
            wk_sb.append(t)
        # wo prefetch: emitted here so it sits early on the sync queue; its
        # slot-wait resolves as soon as the wv tiles retire after V-proj
        wo_sb = []
        for ch in range(NCH):
            t = wpool.tile([128, D], F32R, tag="w")
            nc.sync.dma_start(out=t, in_=wo3[ch])
            wo_sb.append(t)

        # Software pipeline over heads: head h's context matmuls are emitted
        # one head later, so PE fills the DVE-add -> ACT-exp latency of head
        # h with head h+1's scores (and the next chunk's projections) and
        # never idles long enough for HAM to re-throttle.
        def emit_scores(h):
            i, p0 = h // 2, (h % 2) * 64
            amt_h = amtpool.tile([128, SB, S], F16, tag="amt")
            # GpSimd (SWDGE) queue: keeps amt streams off the sync queue so
            # weight prefetch (esp. wo) isn't stuck behind them, and off the
            # busy compute engines' FIFOs (GpSimd is otherwise nearly idle)
            nc.gpsimd.dma_start(out=amt_h, in_=amt4[h])
            QT_h = QT_sb[p0:p0 + 64, i, :]
            attn_tiles = []
            for kb in range(SB):
                ps = ps_sc.tile([128, 512], F32, tag="sc")
                nc.tensor.matmul(
                    ps, lhsT=KT_sb[p0:p0 + 64, i, kb * 128:(kb + 1) * 128],
                    rhs=QT_h, start=True, stop=True,
                )
                nc.vector.tensor_add(ps, ps, amt_h[:, kb, :])
                at = attnpool.tile([128, 512], F32R, tag="attn")
                nc.scalar.activation(at, ps, mybir.ActivationFunctionType.Exp)
                attn_tiles.append(at)
            return attn_tiles

        def emit_ctx(h, attn_tiles):
            i, p0 = h // 2, (h % 2) * 64
            pc = ps_ctx.tile([DK + 1, 512], F32, tag="ctx")
            for kb in range(SB):
                nc.tensor.matmul(
                    pc, lhsT=V_sb[:, kb, h * 65:(h + 1) * 65],
                    rhs=attn_tiles[kb], start=(kb == 0), stop=(kb == SB - 1),
                )
            # custom-DVE reciprocal can't read PSUM on HW; stage sums in SBUF
            sums_sb = rcpool.tile([1, 512], F32, tag="recip")
            nc.scalar.copy(sums_sb, pc[DK:DK + 1, :])
            recip_f32 = rcpool.tile([1, 512], F32, tag="recip")
            nc.vector.reciprocal_approx_fast(out=recip_f32, in_=sums_sb)
            recip = rc1pool.tile([1, 512], F32R, tag="recip_r")
            nc.scalar.copy(recip, recip_f32)
            # broadcast 1/denom across 64 partitions via a K=1 matmul
            # (GpSimd's queue is reserved for the amt DMA stream)
            pr = ps_r.tile([64, 512], F32, tag="r")
            nc.tensor.matmul(pr, lhsT=ones_sb[:, :64], rhs=recip,
                             start=True, stop=True)
            r_sb = rc1pool.tile([64, 512], F32, tag="rbc")
            nc.scalar.copy(r_sb, pr)
            nc.vector.tensor_mul(ctxT_sb[p0:p0 + 64, i, :], pc[0:DK, :], r_sb)

        pending = None  # (head, attn_tiles) awaiting its context matmuls
        for i in range(NCH):  # e-chunk i covers heads 2i, 2i+1
            pq = ps_proj.tile([128, 512], F32, tag="proj")
            for dc in range(NCH):
                nc.tensor.matmul(
                    pq, lhsT=wq_sb[dc][:, i * 128:(i + 1) * 128],
                    rhs=qT_sb[:, dc, :],
                    start=(dc == 0), stop=(dc == NCH - 1),
                )
            nc.scalar.add(QT_sb[:, i, :], pq, bq_sb[:, i:i + 1])
            pk = ps_proj.tile([128, 512], F32, tag="proj")
            for dc in range(NCH):
                nc.tensor.matmul(
                    pk, lhsT=wk_sb[dc][:, i * 128:(i + 1) * 128],
                    rhs=kT_sb[:, dc, :],
                    start=(dc == 0), stop=(dc == NCH - 1),
                )
            nc.scalar.add(KT_sb[:, i, :], pk, bk_sb[:, i:i + 1])

            for sub in range(2):
                h = 2 * i + sub
                tiles = emit_scores(h)
                if pending is not None:
                    emit_ctx(*pending)
                pending = (h, tiles)
        emit_ctx(*pending)

        # ---- output projection: out_rev[q', e] = ctxT^T @ woT + b_o ----
        for sb in range(SB):
            for eh in range(2):
                po = ps_proj.tile([128, 512], F32, tag="proj")
                for ch in range(NCH):
                    nc.tensor.matmul(
                        po, lhsT=ctxT_sb[:, ch, sb * 128:(sb + 1) * 128],
                        rhs=wo_sb[ch][:, eh * 512:(eh + 1) * 512],
                        start=(ch == 0), stop=False,
                    )
                nc.tensor.matmul(
                    po, lhsT=ones_sb[:, :128],
                    rhs=bor_sb[:, eh * 512:(eh + 1) * 512],
                    start=False, stop=True,
                )
                osb = outpool.tile([128, 512], F32, tag="out")
                nc.scalar.copy(osb, po)
                nc.sync.dma_start(
                    out=out3[sb, :, eh * 512:(eh + 1) * 512], in_=osb)

    nc.compile()
    return nc


def _round_f32r(a):
    """Round-to-nearest to fp32r's 19-bit (1+8+13... wait — explicit 13-bit)
    mantissa so the PE's fp32r truncation is lossless on these operands."""
    b = np.ascontiguousarray(a, np.float32).view(np.uint32)
    return ((b + 0x1000) & np.uint32(0xFFFFE000)).view(np.float32)


def _prep_inputs(query, key, value, mask, w_q, b_q, w_k, b_k, w_v, b_v,
                 w_o, b_o, rel_bias):
    query = np.asarray(query, np.float32)
    key = np.asarray(key, np.float32)
    value = np.asarray(value, np.float32)
    mask = np.asarray(mask)
    w_q = np.asarray(w_q, np.float32)
    w_k = np.asarray(w_k, np.float32)
    w_v = np.asarray(w_v, np.float32)
    w_o = np.asarray(w_o, np.float32)
    b_q = np.asarray(b_q, np.float32)
    b_k = np.asarray(b_k, np.float32)
    b_v = np.asarray(b_v, np.float32)
    b_o = np.asarray(b_o, np.float32)
    rel_bias = np.asarray(rel_bias, np.float32)

    shared = {
        "wq": _round_f32r(w_q.T / 8.0),
        "wk": _round_f32r(w_k.T),
        "wv": _round_f32r(w_v.T),
        "wo": _round_f32r(w_o.T),
        "bq": np.ascontiguousarray((b_q / 8.0).reshape(NCH, 128).T),
        "bk": np.ascontiguousarray(b_k.reshape(NCH, 128).T),
        "bvr": _round_f32r(b_v.reshape(1, D)),
        "bor": _round_f32r(b_o.reshape(1, D)),
    }

    # biasT_rev[h, k, q'] = rel_bias[k + q', h]
    idx = np.arange(S)[:, None] + np.arange(S)[None, :]  # [k, q'] in [0, 1022]
    bias_t = rel_bias[idx]                 # [S, S, H]
    bias_t = np.ascontiguousarray(bias_t.transpose(2, 0, 1))  # [H, k, q']

    in_maps = []
    for c in range(N_CORES):
        # maskT_rev[k, q'] additive: mask[c, 0, 511-q', k] == 0 -> MASK_NEG
        m = mask[c, 0][::-1, :].T          # [k, q'] values in {0, 1}
        madd = np.where(m == 0, np.float32(MASK_NEG), np.float32(0.0))
        amt = (bias_t + madd[None]).astype(np.float16)
        im = dict(shared)
        im["qT"] = _round_f32r(query[c].T[:, ::-1])
        im["kT"] = _round_f32r(key[c].T)
        im["vT"] = _round_f32r(value[c].T)
        im["amt"] = np.ascontiguousarray(amt)
        in_maps.append(im)
    return in_maps


def kernel(query, key, value, mask, w_q, b_q, w_k, b_k, w_v, b_v, w_o, b_o,
           rel_bias, _run_opts=None):
    if "nc" not in _CACHE:
        _CACHE["nc"] = _build_program()
    nc = _CACHE["nc"]
    in_maps = _prep_inputs(query, key, value, mask, w_q, b_q, w_k, b_k,
                           w_v, b_v, w_o, b_o, rel_bias)
    opts = _run_opts or {}
    res = run_bass_kernel_spmd(nc, in_maps, list(range(N_CORES)), **opts)
    out = np.stack([res.results[c]["out"][::-1, :] for c in range(N_CORES)])
    if _run_opts is not None:
        _CACHE["last_result"] = res
    return out.astype(np.float32)



# revision 2
# speedup vs baseline: 1.3765x; 1.3765x over previous
"""MultiHeadAttention Trainium2 kernel (8 NeuronCores, data-parallel over batch).

Contract: kernel(**inputs) takes the FULL inputs from setup_inputs() and
returns the FULL [8, 512, 1024] output. Batch element c runs on NeuronCore c
(B == n_cores == 8); each core runs the same Bass/Tile program on its own
shard. No collectives.

Per-core computation (batch b, S=512, D=1024, H=16, Dk=64), all matmul
operands bf16 (fp32 PSUM accumulation), which halves HBM traffic and SBUF
footprint vs fp32r at the same PE rate:
  QT = (w_q/8)^T-proj of query^T  -> [D, S]  (query columns reversed)
  KT likewise (unscaled)          -> [D, S]
  V  = value proj + b_v           -> [128, sb, h, 65]  (col 64 = ones)
  per head h (chunk i = h//2, partitions p0 = (h%2)*64):
    scoresT[k,q'] = KT_h^T @ QT_h           (2 matmuls per [128,1024] PSUM)
    expS = exp(scoresT)                     (ACT, [128,1024] granularity)
    attn = expS * emt_h                     (DVE 2x bf16; emt = exp(bias)*mask
                                             precomputed on host, fp16)
    ctxT[65,S] = [V_h | 1]^T @ attn         (row 64 = softmax denominators)
    recip = 1/denom (DVE) -> bcast to 64 partitions (GpSimd) -> ctxT *= recip
  out_rev[q', e] = ctxT^T @ w_o^T + b_o ; host un-reverses rows.

The query-direction reversal makes rel_bias[k - q + 511, h] == rel_bias[k +
q', h], a positive-stride layout the host materializes (as exp) directly.
Bias adds ride the PSUM->SBUF drains: b_q/b_k as DVE tensor_scalar adds,
b_v/b_o as DVE tensor_tensor adds against partition-broadcast rows.
"""
import numpy as np
import ml_dtypes

import concourse.bass as bass
import concourse.tile as tile
from concourse import bacc, library_config, mybir
from concourse.bass_utils import run_bass_kernel_spmd

S = 512
D = 1024
H = 16
DK = 64
N_CORES = 8
NCH = D // 128  # 8 d-model chunks of 128
SB = S // 128   # 4 seq blocks of 128
F32 = mybir.dt.float32
BF16 = mybir.dt.bfloat16
F16 = mybir.dt.float16

BF = ml_dtypes.bfloat16

_CACHE = {}


def _build_program():
    nc = bacc.Bacc("TRN2", target_bir_lowering=False, debug=False,
                   num_devices=N_CORES)

    qT = nc.dram_tensor("qT", [D, S], BF16, kind="ExternalInput").ap()
    kT = nc.dram_tensor("kT", [D, S], BF16, kind="ExternalInput").ap()
    vT = nc.dram_tensor("vT", [D, S], BF16, kind="ExternalInput").ap()
    emt = nc.dram_tensor("emt", [H, S, S], F16, kind="ExternalInput").ap()
    wq = nc.dram_tensor("wq", [D, D], BF16, kind="ExternalInput").ap()
    wk = nc.dram_tensor("wk", [D, D], BF16, kind="ExternalInput").ap()
    wv = nc.dram_tensor("wv", [D, D], BF16, kind="ExternalInput").ap()
    wo = nc.dram_tensor("wo", [D, D], BF16, kind="ExternalInput").ap()
    bq = nc.dram_tensor("bq", [128, NCH], F32, kind="ExternalInput").ap()
    bk = nc.dram_tensor("bk", [128, NCH], F32, kind="ExternalInput").ap()
    bve = nc.dram_tensor("bve", [1, D], BF16, kind="ExternalInput").ap()
    boe = nc.dram_tensor("boe", [1, D], BF16, kind="ExternalInput").ap()
    out = nc.dram_tensor("out", [S, D], BF16, kind="ExternalOutput").ap()

    qT3 = qT.rearrange("(c p) s -> p c s", p=128)      # [128, 8, 512]
    kT3 = kT.rearrange("(c p) s -> p c s", p=128)
    vT3 = vT.rearrange("(c p) s -> p c s", p=128)
    emt4 = emt.rearrange("h (kb p) q -> h p kb q", p=128)  # [16, 128, 4, 512]
    wq3 = wq.rearrange("(c p) e -> c p e", p=128)      # [8, 128, 1024]
    wk3 = wk.rearrange("(c p) e -> c p e", p=128)
    wv3 = wv.rearrange("(c p) e -> c p e", p=128)
    wo3 = wo.rearrange("(c p) e -> c p e", p=128)
    out3 = out.rearrange("(sb p) e -> sb p e", p=128)  # [4, 128, 1024]

    from contextlib import ExitStack

    with tile.TileContext(nc) as tc, ExitStack() as ctx:
        singles = ctx.enter_context(tc.tile_pool(name="singles", bufs=1))
        wpool = ctx.enter_context(tc.tile_pool(name="wpool", bufs=32))
        emtpool = ctx.enter_context(tc.tile_pool(name="emtpool", bufs=3))
        exppool = ctx.enter_context(tc.tile_pool(name="exppool", bufs=2))
        attnpool = ctx.enter_context(tc.tile_pool(name="attnpool", bufs=3))
        smallpool = ctx.enter_context(tc.tile_pool(name="smallpool", bufs=2))
        outpool = ctx.enter_context(tc.tile_pool(name="outpool", bufs=2))
        ps_proj = ctx.enter_context(tc.tile_pool(name="ps_proj", bufs=2, space="PSUM"))
        ps_sc = ctx.enter_context(tc.tile_pool(name="ps_sc", bufs=2, space="PSUM"))
        ps_ctx = ctx.enter_context(tc.tile_pool(name="ps_ctx", bufs=2, space="PSUM"))

        # partition_broadcast is a GpSimd extended instruction (attn library)
        nc.gpsimd.load_library(library_config.attn)

        # ---- constants ----
        bq_sb = singles.tile([128, NCH], F32, tag="bq")
        bk_sb = singles.tile([128, NCH], F32, tag="bk")
        nc.sync.dma_start(out=bq_sb, in_=bq)
        nc.sync.dma_start(out=bk_sb, in_=bk)
        bve_sb = singles.tile([1, D], BF16, tag="bve")
        boe_sb = singles.tile([1, D], BF16, tag="boe")
        nc.sync.dma_start(out=bve_sb, in_=bve)
        nc.sync.dma_start(out=boe_sb, in_=boe)
        ones_bf = singles.tile([1, 512], BF16, tag="ones")
        nc.vector.memset(ones_bf, 1.0)
        bve_bc = singles.tile([128, D], BF16, tag="bve_bc")
        boe_bc = singles.tile([128, D], BF16, tag="boe_bc")
        nc.gpsimd.partition_broadcast(bve_bc, bve_sb)
        nc.gpsimd.partition_broadcast(boe_bc, boe_sb)
        # pre-load the ACT exp table before the first real exp
        exp_warm = singles.tile([1, 32], F32, tag="exp_warm")
        nc.scalar.activation(exp_warm, ones_bf[:, 0:32],
                             mybir.ActivationFunctionType.Exp)

        # HAM warm-up: ~5us of throwaway matmuls while input DMAs stream, so
        # the PE clock-gate reaches 8/8 before real work is ready. Operands
        # are memset on-chip so no DMA gates the first matmul.
        for _ in range(14):
            pd = ps_proj.tile([128, 512], F32, tag="proj")
            nc.tensor.matmul(pd, lhsT=ones_bf[:, :128], rhs=ones_bf,
                             start=True, stop=True)

        # ---- bulk loads, in consumption order (sync queue is FIFO) ----
        wv_sb = []
        for dc in range(NCH):
            t = wpool.tile([128, D], BF16, tag="w")
            nc.sync.dma_start(out=t, in_=wv3[dc])
            wv_sb.append(t)
        vT_sb = singles.tile([128, NCH, S], BF16, tag="vT")
        nc.sync.dma_start(out=vT_sb, in_=vT3)
        wq_sb = []
        for dc in range(NCH):
            t = wpool.tile([128, D], BF16, tag="w")
            nc.sync.dma_start(out=t, in_=wq3[dc])
            wq_sb.append(t)
        qT_sb = singles.tile([128, NCH, S], BF16, tag="qT")
        nc.sync.dma_start(out=qT_sb, in_=qT3)
        wk_sb = []
        for dc in range(NCH):
            t = wpool.tile([128, D], BF16, tag="w")
            nc.sync.dma_start(out=t, in_=wk3[dc])
            wk_sb.append(t)
        kT_sb = singles.tile([128, NCH, S], BF16, tag="kT")
        nc.sync.dma_start(out=kT_sb, in_=kT3)
        wo_sb = []
        for dc in range(NCH):
            t = wpool.tile([128, D], BF16, tag="w")
            nc.sync.dma_start(out=t, in_=wo3[dc])
            wo_sb.append(t)

        # emt per head on the GpSimd (SWDGE) queue, off the sync queue
        emt_sb = []
        for h in range(H):
            t = emtpool.tile([128, SB, S], F16, tag="emt")
            nc.gpsimd.dma_start(out=t, in_=emt4[h])
            emt_sb.append(t)

        # ---- persistent activations ----
        QT_sb = singles.tile([128, NCH, S], BF16, tag="QT")
        KT_sb = singles.tile([128, NCH, S], BF16, tag="KT")
        V_sb = singles.tile([128, SB, H, DK + 1], BF16, tag="V")
        ctxT_sb = singles.tile([128, NCH, S], BF16, tag="ctxT")
        nc.vector.memset(V_sb[:, :, :, DK:DK + 1], 1.0)

        # ---- V projection: V[s, e] = vT^T @ wv + b_v ----
        for sb in range(SB):
            for eh in range(2):
                pv = ps_proj.tile([128, 512], F32, tag="proj")
                for dc in range(NCH):
                    nc.tensor.matmul(
                        pv,
                        lhsT=vT_sb[:, dc, sb * 128:(sb + 1) * 128],
                        rhs=wv_sb[dc][:, eh * 512:(eh + 1) * 512],
                        start=(dc == 0), stop=(dc == NCH - 1),
                    )
                nc.vector.tensor_add(
                    V_sb[:, sb, 8 * eh:8 * eh + 8, 0:DK],
                    pv.rearrange("p (h d) -> p h d", d=DK),
                    bve_bc[:, eh * 512:(eh + 1) * 512].rearrange(
                        "p (h d) -> p h d", d=DK),
                )

        # ---- interleaved Q/K projection chunks + attention heads ----
        def emit_scores_exp(h):
            """Scores + exp for head h at [128,1024] granularity."""
            i, p0 = h // 2, (h % 2) * 64
            QT_h = QT_sb[p0:p0 + 64, i, :]
            exp_t = exppool.tile([128, SB, S], BF16, tag="exp")
            for half in range(2):
                psc = ps_sc.tile([128, 1024], F32, tag="sc")
                for j in range(2):
                    kb = 2 * half + j
                    nc.tensor.matmul(
                        psc[:, j * 512:(j + 1) * 512],
                        lhsT=KT_sb[p0:p0 + 64, i, kb * 128:(kb + 1) * 128],
                        rhs=QT_h, start=True, stop=True,
                    )
                nc.scalar.activation(
                    exp_t[:, 2 * half:2 * half + 2, :],
                    psc.rearrange("p (a q) -> p a q", q=512),
                    mybir.ActivationFunctionType.Exp,
                )
            attn_t = attnpool.tile([128, SB, S], BF16, tag="attn")
            nc.vector.tensor_mul(attn_t, exp_t, emt_sb[h])
            return attn_t

        def emit_ctx(h, attn_t):
            i, p0 = h // 2, (h % 2) * 64
            pc = ps_ctx.tile([DK + 1, 512], F32, tag="ctx")
            for kb in range(SB):
                nc.tensor.matmul(
                    pc, lhsT=V_sb[:, kb, h, :], rhs=attn_t[:, kb, :],
                    start=(kb == 0), stop=(kb == SB - 1),
                )
            # DVE custom reciprocal can't read PSUM; stage sums in SBUF
            sums_sb = smallpool.tile([1, 512], F32, tag="sums")
            nc.scalar.copy(sums_sb, pc[DK:DK + 1, :])
            rc = smallpool.tile([1, 512], F32, tag="rc")
            nc.vector.reciprocal_approx_fast(out=rc, in_=sums_sb)
            rbc = smallpool.tile([64, 512], F32, tag="rbc")
            nc.gpsimd.partition_broadcast(rbc, rc)
            nc.vector.tensor_mul(ctxT_sb[p0:p0 + 64, i, :], pc[0:DK, :], rbc)

        pending = None
        for i in range(NCH):  # e-chunk i covers heads 2i, 2i+1
            pq = ps_proj.tile([128, 512], F32, tag="proj")
            for dc in range(NCH):
                nc.tensor.matmul(
                    pq, lhsT=wq_sb[dc][:, i * 128:(i + 1) * 128],
                    rhs=qT_sb[:, dc, :],
                    start=(dc == 0), stop=(dc == NCH - 1),
                )
            nc.vector.tensor_scalar_add(QT_sb[:, i, :], pq, bq_sb[:, i:i + 1])
            pk = ps_proj.tile([128, 512], F32, tag="proj")
            for dc in range(NCH):
                nc.tensor.matmul(
                    pk, lhsT=wk_sb[dc][:, i * 128:(i + 1) * 128],
                    rhs=kT_sb[:, dc, :],
                    start=(dc == 0), stop=(dc == NCH - 1),
                )
            nc.vector.tensor_scalar_add(KT_sb[:, i, :], pk, bk_sb[:, i:i + 1])

            for sub in range(2):
                h = 2 * i + sub
                attn_t = emit_scores_exp(h)
                if pending is not None:
                    emit_ctx(*pending)
                pending = (h, attn_t)
        emit_ctx(*pending)

        # keep the PE clock-gate open across the attention->out-proj seam
        for _ in range(6):
            pd = ps_proj.tile([128, 512], F32, tag="proj")
            nc.tensor.matmul(pd, lhsT=ones_bf[:, :128], rhs=ones_bf,
                             start=True, stop=True)

        # ---- output projection: out_rev[q', e] = ctxT^T @ wo + b_o ----
        for sb in range(SB):
            for eh in range(2):
                po = ps_proj.tile([128, 512], F32, tag="proj")
                for ch in range(NCH):
                    nc.tensor.matmul(
                        po, lhsT=ctxT_sb[:, ch, sb * 128:(sb + 1) * 128],
                        rhs=wo_sb[ch][:, eh * 512:(eh + 1) * 512],
                        start=(ch == 0), stop=(ch == NCH - 1),
                    )
                osb = outpool.tile([128, 512], BF16, tag="out")
                nc.vector.tensor_add(osb, po, boe_bc[:, eh * 512:(eh + 1) * 512])
                nc.sync.dma_start(
                    out=out3[sb, :, eh * 512:(eh + 1) * 512], in_=osb)

    nc.compile()
    return nc


def _prep_inputs(query, key, value, mask, w_q, b_q, w_k, b_k, w_v, b_v,
                 w_o, b_o, rel_bias):
    query = np.asarray(query, np.float32)
    key = np.asarray(key, np.float32)
    value = np.asarray(value, np.float32)
    mask = np.asarray(mask)
    w_q = np.asarray(w_q, np.float32)
    w_k = np.asarray(w_k, np.float32)
    w_v = np.asarray(w_v, np.float32)
    w_o = np.asarray(w_o, np.float32)
    b_q = np.asarray(b_q, np.float32)
    b_k = np.asarray(b_k, np.float32)
    b_v = np.asarray(b_v, np.float32)
    b_o = np.asarray(b_o, np.float32)
    rel_bias = np.asarray(rel_bias, np.float32)

    shared = {
        "wq": np.ascontiguousarray((w_q.T / 8.0).astype(BF)),
        "wk": np.ascontiguousarray(w_k.T.astype(BF)),
        "wv": np.ascontiguousarray(w_v.T.astype(BF)),
        "wo": np.ascontiguousarray(w_o.T.astype(BF)),
        "bq": np.ascontiguousarray((b_q / 8.0).reshape(NCH, 128).T),
        "bk": np.ascontiguousarray(b_k.reshape(NCH, 128).T),
        "bve": b_v.reshape(1, D).astype(BF),
        "boe": b_o.reshape(1, D).astype(BF),
    }

    # ebias[h, k, q'] = exp(rel_bias[k + q', h]) ; masked entries -> 0
    idx = np.arange(S)[:, None] + np.arange(S)[None, :]   # [k, q'] in [0, 1022]
    ebias = np.exp(rel_bias[idx])                          # [S, S, H]
    ebias = np.ascontiguousarray(ebias.transpose(2, 0, 1))  # [H, k, q']

    in_maps = []
    for c in range(N_CORES):
        m01 = mask[c, 0][::-1, :].T.astype(np.float32)     # [k, q'] in {0,1}
        emt = (ebias * m01[None]).astype(np.float16)
        im = dict(shared)
        im["qT"] = np.ascontiguousarray(query[c].T[:, ::-1].astype(BF))
        im["kT"] = np.ascontiguousarray(key[c].T.astype(BF))
        im["vT"] = np.ascontiguousarray(value[c].T.astype(BF))
        im["emt"] = np.ascontiguousarray(emt)
        in_maps.append(im)
    return in_maps


def kernel(query, key, value, mask, w_q, b_q, w_k, b_k, w_v, b_v, w_o, b_o,
           rel_bias, _run_opts=None):
    if "nc" not in _CACHE:
        _CACHE["nc"] = _build_program()
    nc = _CACHE["nc"]
    in_maps = _prep_inputs(query, key, value, mask, w_q, b_q, w_k, b_k,
                           w_v, b_v, w_o, b_o, rel_bias)
    opts = _run_opts or {}
    res = run_bass_kernel_spmd(nc, in_maps, list(range(N_CORES)), **opts)
    out = np.stack([np.asarray(res.results[c]["out"], np.float32)[::-1, :]
                    for c in range(N_CORES)])
    if _run_opts is not None:
        _CACHE["last_result"] = res
    return out


# revision 9
# speedup vs baseline: 1.6247x; 1.1803x over previous
"""MultiHeadAttention Trainium2 kernel (8 NeuronCores, data-parallel over batch).

Contract: kernel(**inputs) takes the FULL inputs from setup_inputs() and
returns the FULL [8, 512, 1024] output. Batch element c runs on NeuronCore c
(B == n_cores == 8); each core runs the same Bass/Tile program on its own
shard. No collectives.

Per-core computation (batch b, S=512, D=1024, H=16, Dk=64), all matmul
operands bf16 (fp32 PSUM accumulation), which halves HBM traffic and SBUF
footprint vs fp32r at the same PE rate:
  QT = (w_q/8)^T-proj of query^T  -> [D, S]  (query columns reversed)
  KT likewise (unscaled)          -> [D, S]
  V  = value proj + b_v           -> [128, sb, h, 65]  (col 64 = ones)
  per head h (chunk i = h//2, partitions p0 = (h%2)*64):
    scoresT[k,q'] = KT_h^T @ QT_h           (2 matmuls per [128,1024] PSUM)
    expS = exp(scoresT)                     (ACT, [128,1024] granularity)
    attn = expS * emt_h                     (DVE 2x bf16; emt = exp(bias)*mask
                                             precomputed on host, fp16)
    ctxT[65,S] = [V_h | 1]^T @ attn         (row 64 = softmax denominators)
    recip = 1/denom (DVE) -> bcast to 64 partitions (GpSimd) -> ctxT *= recip
  out_rev[q', e] = ctxT^T @ w_o^T + b_o ; host un-reverses rows.

The query-direction reversal makes rel_bias[k - q + 511, h] == rel_bias[k +
q', h], a positive-stride layout the host materializes (as exp) directly.
Bias adds ride the PSUM->SBUF drains: b_q/b_k as DVE tensor_scalar adds,
b_v/b_o as DVE tensor_tensor adds against partition-broadcast rows.
"""
import numpy as np
import ml_dtypes

import concourse.bass as bass
import concourse.tile as tile
from concourse import bacc, library_config, mybir
from concourse.bass_utils import run_bass_kernel_spmd

S = 512
D = 1024
H = 16
DK = 64
N_CORES = 8
NCH = D // 128  # 8 d-model chunks of 128
SB = S // 128   # 4 seq blocks of 128
F32 = mybir.dt.float32
BF16 = mybir.dt.bfloat16
F16 = mybir.dt.float16

BF = ml_dtypes.bfloat16

_CACHE = {}


def _build_program():
    nc = bacc.Bacc("TRN2", target_bir_lowering=False, debug=False,
                   num_devices=N_CORES)

    qT = nc.dram_tensor("qT", [D, S], BF16, kind="ExternalInput").ap()
    kT = nc.dram_tensor("kT", [D, S], BF16, kind="ExternalInput").ap()
    vT = nc.dram_tensor("vT", [D, S], BF16, kind="ExternalInput").ap()
    emt = nc.dram_tensor("emt", [H, S, S], F16, kind="ExternalInput").ap()
    wq = nc.dram_tensor("wq", [D, D], BF16, kind="ExternalInput").ap()
    wk = nc.dram_tensor("wk", [D, D], BF16, kind="ExternalInput").ap()
    wv = nc.dram_tensor("wv", [D, D], BF16, kind="ExternalInput").ap()
    wo = nc.dram_tensor("wo", [D, D], BF16, kind="ExternalInput").ap()
    bq = nc.dram_tensor("bq", [128, NCH], F32, kind="ExternalInput").ap()
    bk = nc.dram_tensor("bk", [128, NCH], F32, kind="ExternalInput").ap()
    bve = nc.dram_tensor("bve", [1, D], BF16, kind="ExternalInput").ap()
    boe = nc.dram_tensor("boe", [1, D], BF16, kind="ExternalInput").ap()
    out = nc.dram_tensor("out", [S, D], BF16, kind="ExternalOutput").ap()

    qT3 = qT.rearrange("(c p) s -> p c s", p=128)      # [128, 8, 512]
    kT3 = kT.rearrange("(c p) s -> p c s", p=128)
    vT3 = vT.rearrange("(c p) s -> p c s", p=128)
    emt4 = emt.rearrange("h (kb p) q -> h p kb q", p=128)  # [16, 128, 4, 512]
    wq3 = wq.rearrange("(c p) e -> c p e", p=128)      # [8, 128, 1024]
    wk3 = wk.rearrange("(c p) e -> c p e", p=128)
    wv3 = wv.rearrange("(c p) e -> c p e", p=128)
    wo3 = wo.rearrange("(c p) e -> c p e", p=128)
    out3 = out.rearrange("(sb p) e -> sb p e", p=128)  # [4, 128, 1024]

    from contextlib import ExitStack

    with tile.TileContext(nc) as tc, ExitStack() as ctx:
        singles = ctx.enter_context(tc.tile_pool(name="singles", bufs=1))
        wpool = ctx.enter_context(tc.tile_pool(name="wpool", bufs=4))
        emtpool = ctx.enter_context(tc.tile_pool(name="emtpool", bufs=3))
        exppool = ctx.enter_context(tc.tile_pool(name="exppool", bufs=2))
        attnpool = ctx.enter_context(tc.tile_pool(name="attnpool", bufs=3))
        smallpool = ctx.enter_context(tc.tile_pool(name="smallpool", bufs=2))
        outpool = ctx.enter_context(tc.tile_pool(name="outpool", bufs=2))
        ps_proj = ctx.enter_context(tc.tile_pool(name="ps_proj", bufs=2, space="PSUM"))
        ps_sc = ctx.enter_context(tc.tile_pool(name="ps_sc", bufs=2, space="PSUM"))
        ps_ctx = ctx.enter_context(tc.tile_pool(name="ps_ctx", bufs=2, space="PSUM"))

        # partition_broadcast is a GpSimd extended instruction (attn library)
        nc.gpsimd.load_library(library_config.attn)

        # ---- constants (scalar-engine DMA queue; keeps the sync queue free
        # for the bulk weight streams whose latency gates the PE) ----
        bq_sb = singles.tile([128, NCH], F32, tag="bq")
        bk_sb = singles.tile([128, NCH], F32, tag="bk")
        nc.scalar.dma_start(out=bq_sb, in_=bq)
        nc.scalar.dma_start(out=bk_sb, in_=bk)
        bve_sb = singles.tile([1, D], BF16, tag="bve")
        boe_sb = singles.tile([1, D], BF16, tag="boe")
        nc.scalar.dma_start(out=bve_sb, in_=bve)
        nc.scalar.dma_start(out=boe_sb, in_=boe)
        ones_bf = singles.tile([1, 512], BF16, tag="ones")
        nc.vector.memset(ones_bf, 1.0)
        bve_bc = singles.tile([128, D], BF16, tag="bve_bc")
        boe_bc = singles.tile([128, D], BF16, tag="boe_bc")
        nc.gpsimd.partition_broadcast(bve_bc, bve_sb)
        nc.gpsimd.partition_broadcast(boe_bc, boe_sb)
        # pre-load the ACT exp table before the first real exp
        exp_warm = singles.tile([1, 32], F32, tag="exp_warm")
        nc.scalar.activation(exp_warm, ones_bf[:, 0:32],
                             mybir.ActivationFunctionType.Exp)

        # HAM warm-up: throwaway matmuls covering the wv+vT DMA window, so
        # the PE clock-gate reaches 8/8 before real work is ready and the PE
        # is never idle long enough to re-throttle. Operands are memset
        # on-chip so no DMA gates the first matmul.
        for _ in range(36):
            pd = ps_proj.tile([128, 512], F32, tag="proj")
            nc.tensor.matmul(pd, lhsT=ones_bf[:, :128], rhs=ones_bf,
                             start=True, stop=True)

        # ---- bulk loads: one DMA per matrix, in consumption order (sync
        # queue is FIFO; few big DMAs beat many small ones on SP issue) ----
        wq4 = wq.rearrange("(c p) e -> p c e", p=128)
        wk4 = wk.rearrange("(c p) e -> p c e", p=128)
        wv4 = wv.rearrange("(c p) e -> p c e", p=128)
        wo4 = wo.rearrange("(c p) e -> p c e", p=128)
        wv_t = wpool.tile([128, NCH, D], BF16, tag="w")
        nc.sync.dma_start(out=wv_t, in_=wv4)
        vT_sb = singles.tile([128, NCH, S], BF16, tag="vT")
        nc.sync.dma_start(out=vT_sb, in_=vT3)
        wq_t = wpool.tile([128, NCH, D], BF16, tag="w")
        nc.sync.dma_start(out=wq_t, in_=wq4)
        qT_sb = singles.tile([128, NCH, S], BF16, tag="qT")
        nc.sync.dma_start(out=qT_sb, in_=qT3)
        wk_t = wpool.tile([128, NCH, D], BF16, tag="w")
        nc.sync.dma_start(out=wk_t, in_=wk4)
        kT_sb = singles.tile([128, NCH, S], BF16, tag="kT")
        nc.sync.dma_start(out=kT_sb, in_=kT3)
        wo_t = wpool.tile([128, NCH, D], BF16, tag="w")
        nc.sync.dma_start(out=wo_t, in_=wo4)
        wv_sb = [wv_t[:, dc, :] for dc in range(NCH)]
        wq_sb = [wq_t[:, dc, :] for dc in range(NCH)]
        wk_sb = [wk_t[:, dc, :] for dc in range(NCH)]
        wo_sb = [wo_t[:, dc, :] for dc in range(NCH)]

        # emt per head, also on the sync queue, emission interleaved with the
        # head loop so slot-waits never sit ahead of anything time-critical
        emt_sb = {}

        def emit_emt(h):
            if h < H:
                t = emtpool.tile([128, SB, S], F16, tag="emt")
                nc.sync.dma_start(out=t, in_=emt4[h])
                emt_sb[h] = t

        for h in range(3):
            emit_emt(h)

        # ---- persistent activations ----
        QT_sb = singles.tile([128, NCH, S], BF16, tag="QT")
        KT_sb = singles.tile([128, NCH, S], BF16, tag="KT")
        V_sb = singles.tile([128, SB, H, DK + 1], BF16, tag="V")
        ctxT_sb = singles.tile([128, NCH, S], BF16, tag="ctxT")
        nc.vector.memset(V_sb[:, :, :, DK:DK + 1], 1.0)

        # ---- V projection: V[s, e] = vT^T @ wv + b_v ----
        for sb in range(SB):
            for eh in range(2):
                pv = ps_proj.tile([128, 512], F32, tag="proj")
                for dc in range(NCH):
                    nc.tensor.matmul(
                        pv,
                        lhsT=vT_sb[:, dc, sb * 128:(sb + 1) * 128],
                        rhs=wv_sb[dc][:, eh * 512:(eh + 1) * 512],
                        start=(dc == 0), stop=(dc == NCH - 1),
                    )
                nc.vector.tensor_add(
                    V_sb[:, sb, 8 * eh:8 * eh + 8, 0:DK],
                    pv.rearrange("p (h d) -> p h d", d=DK),
                    bve_bc[:, eh * 512:(eh + 1) * 512].rearrange(
                        "p (h d) -> p h d", d=DK),
                )

        # ---- interleaved Q/K projection chunks + attention heads ----
        def emit_scores_exp(h):
            """Scores + exp for head h at [128,1024] granularity."""
            i, p0 = h // 2, (h % 2) * 64
            QT_h = QT_sb[p0:p0 + 64, i, :]
            exp_t = exppool.tile([128, SB, S], BF16, tag="exp")
            for half in range(2):
                psc = ps_sc.tile([128, 1024], F32, tag="sc")
                for j in range(2):
                    kb = 2 * half + j
                    nc.tensor.matmul(
                        psc[:, j * 512:(j + 1) * 512],
                        lhsT=KT_sb[p0:p0 + 64, i, kb * 128:(kb + 1) * 128],
                        rhs=QT_h, start=True, stop=True,
                    )
                nc.scalar.activation(
                    exp_t[:, 2 * half:2 * half + 2, :],
                    psc.rearrange("p (a q) -> p a q", q=512),
                    mybir.ActivationFunctionType.Exp,
                )
            attn_t = attnpool.tile([128, SB, S], BF16, tag="attn")
            nc.vector.tensor_mul(attn_t, exp_t, emt_sb[h])
            emit_emt(h + 3)
            return attn_t

        def emit_ctx(h, attn_t):
            i, p0 = h // 2, (h % 2) * 64
            pc = ps_ctx.tile([DK + 1, 512], F32, tag="ctx")
            for kb in range(SB):
                nc.tensor.matmul(
                    pc, lhsT=V_sb[:, kb, h, :], rhs=attn_t[:, kb, :],
                    start=(kb == 0), stop=(kb == SB - 1),
                )
            # DVE custom reciprocal can't read PSUM; stage sums in SBUF
            sums_sb = smallpool.tile([1, 512], F32, tag="sums")
            nc.scalar.copy(sums_sb, pc[DK:DK + 1, :])
            rc = smallpool.tile([1, 512], F32, tag="rc")
            nc.vector.reciprocal_approx_fast(out=rc, in_=sums_sb)
            rbc = smallpool.tile([64, 512], F32, tag="rbc")
            nc.gpsimd.partition_broadcast(rbc, rc)
            nc.vector.tensor_mul(ctxT_sb[p0:p0 + 64, i, :], pc[0:DK, :], rbc)

        pending = []
        for i in range(NCH):  # e-chunk i covers heads 2i, 2i+1
            pq = ps_proj.tile([128, 512], F32, tag="proj")
            for dc in range(NCH):
                nc.tensor.matmul(
                    pq, lhsT=wq_sb[dc][:, i * 128:(i + 1) * 128],
                    rhs=qT_sb[:, dc, :],
                    start=(dc == 0), stop=(dc == NCH - 1),
                )
            nc.vector.tensor_scalar_add(QT_sb[:, i, :], pq, bq_sb[:, i:i + 1])
            pk = ps_proj.tile([128, 512], F32, tag="proj")
            for dc in range(NCH):
                nc.tensor.matmul(
                    pk, lhsT=wk_sb[dc][:, i * 128:(i + 1) * 128],
                    rhs=kT_sb[:, dc, :],
                    start=(dc == 0), stop=(dc == NCH - 1),
                )
            nc.vector.tensor_scalar_add(KT_sb[:, i, :], pk, bk_sb[:, i:i + 1])

            for sub in range(2):
                h = 2 * i + sub
                attn_t = emit_scores_exp(h)
                pending.append((h, attn_t))
                # ctx lags 2 heads behind scores: the per-head serial chain
                # (exp -> mul -> ctx -> sums -> recip -> bcast -> norm) is
                # ~2x the per-head PE work, so depth-2 keeps the PE fed
                if len(pending) > 2:
                    emit_ctx(*pending.pop(0))
        while pending:
            emit_ctx(*pending.pop(0))

        # keep the PE clock-gate open across the attention->out-proj seam
        for _ in range(14):
            pd = ps_proj.tile([128, 512], F32, tag="proj")
            nc.tensor.matmul(pd, lhsT=ones_bf[:, :128], rhs=ones_bf,
                             start=True, stop=True)

        # ---- output projection: out_rev[q', e] = ctxT^T @ wo + b_o ----
        for sb in range(SB):
            for eh in range(2):
                po = ps_proj.tile([128, 512], F32, tag="proj")
                for ch in range(NCH):
                    nc.tensor.matmul(
                        po, lhsT=ctxT_sb[:, ch, sb * 128:(sb + 1) * 128],
                        rhs=wo_sb[ch][:, eh * 512:(eh + 1) * 512],
                        start=(ch == 0), stop=(ch == NCH - 1),
                    )
                osb = outpool.tile([128, 512], BF16, tag="out")
                nc.vector.tensor_add(osb, po, boe_bc[:, eh * 512:(eh + 1) * 512])
                nc.sync.dma_start(
                    out=out3[sb, :, eh * 512:(eh + 1) * 512], in_=osb)

    nc.compile()
    return nc


def _prep_inputs(query, key, value, mask, w_q, b_q, w_k, b_k, w_v, b_v,
                 w_o, b_o, rel_bias):
    query = np.asarray(query, np.float32)
    key = np.asarray(key, np.float32)
    value = np.asarray(value, np.float32)
    mask = np.asarray(mask)
    w_q = np.asarray(w_q, np.float32)
    w_k = np.asarray(w_k, np.float32)
    w_v = np.asarray(w_v, np.float32)
    w_o = np.asarray(w_o, np.float32)
    b_q = np.asarray(b_q, np.float32)
    b_k = np.asarray(b_k, np.float32)
    b_v = np.asarray(b_v, np.float32)
    b_o = np.asarray(b_o, np.float32)
    rel_bias = np.asarray(rel_bias, np.float32)

    shared = {
        "wq": np.ascontiguousarray((w_q.T / 8.0).astype(BF)),
        "wk": np.ascontiguousarray(w_k.T.astype(BF)),
        "wv": np.ascontiguousarray(w_v.T.astype(BF)),
        "wo": np.ascontiguousarray(w_o.T.astype(BF)),
        "bq": np.ascontiguousarray((b_q / 8.0).reshape(NCH, 128).T),
        "bk": np.ascontiguousarray(b_k.reshape(NCH, 128).T),
        "bve": b_v.reshape(1, D).astype(BF),
        "boe": b_o.reshape(1, D).astype(BF),
    }

    # ebias[h, k, q'] = exp(rel_bias[k + q', h]) ; masked entries -> 0
    idx = np.arange(S)[:, None] + np.arange(S)[None, :]   # [k, q'] in [0, 1022]
    ebias = np.exp(rel_bias[idx])                          # [S, S, H]
    ebias = np.ascontiguousarray(ebias.transpose(2, 0, 1))  # [H, k, q']

    in_maps = []
    for c in range(N_CORES):
        m01 = mask[c, 0][::-1, :].T.astype(np.float32)     # [k, q'] in {0,1}
        emt = (ebias * m01[None]).astype(np.float16)
        im = dict(shared)
        im["qT"] = np.ascontiguousarray(query[c].T[:, ::-1].astype(BF))
        im["kT"] = np.ascontiguousarray(key[c].T.astype(BF))
        im["vT"] = np.ascontiguousarray(value[c].T.astype(BF))
        im["emt"] = np.ascontiguousarray(emt)
        in_maps.append(im)
    return in_maps


def kernel(query, key, value, mask, w_q, b_q, w_k, b_k, w_v, b_v, w_o, b_o,
           rel_bias, _run_opts=None):
    if "nc" not in _CACHE:
        _CACHE["nc"] = _build_program()
    nc = _CACHE["nc"]
    in_maps = _prep_inputs(query, key, value, mask, w_q, b_q, w_k, b_k,
                           w_v, b_v, w_o, b_o, rel_bias)
    opts = _run_opts or {}
    res = run_bass_kernel_spmd(nc, in_maps, list(range(N_CORES)), **opts)
    out = np.stack([np.asarray(res.results[c]["out"], np.float32)[::-1, :]
                    for c in range(N_CORES)])
    if _run_opts is not None:
        _CACHE["last_result"] = res
    return out
